# revision 26
# baseline (speedup 1.0000x reference)
"""Trainium2 Bass kernel for nn_MultiHeadAttention_35570919146065 (B=2, S=2048,
D=1024, H=16, causal + relative-position bias).

Sharding (8 NeuronCores):
  launch 1: core c = (batch c//4, heads 4*(c%4)..): QKV projections +
            attention, emits attnT [256, 2048] bf16 per core.
  launch 2: core c = (batch c//4, token block c%4 of 512): out projection.

All matmuls bf16 with fp32 PSUM accumulation. Relative-position bias and the
causal mask are folded into one multiplicative term: P = exp(S^T) * ebufT,
where ebias[x] = exp(sum_d rel_table[x, d]) for x <= 2047 and exp(-30) ~= 0
for x > 2047 (masked), x = j - i + 2047. Softmax denominators come from an
appended ones-column in the PV matmul; normalization is deferred to the small
attnT tile.
"""

import numpy as np

import concourse.bass as bass
import concourse.mybir as mybir
import concourse.tile as tile
from concourse import bacc
from concourse.bass_utils import run_bass_kernel_spmd
from concourse.masks import make_identity

B, S, D, H = 2, 2048, 1024, 16
DK = D // H
NCORES = 8
HPC = 4  # heads per core
FPC = HPC * DK  # 256 features per core

QTILE = 512
NQT = S // QTILE
NKC = S // 128
NTT = S // 128  # token tiles

EB_W = 2432  # ebufT free size
EB_LEN = 2560  # ebias length
REL_ROWS = 2048  # rel rows needed (x <= 2047)

DT = mybir.dt
F32 = DT.float32
BF16 = DT.bfloat16
AF = mybir.ActivationFunctionType


def row_bcast(ap, n):
    """Broadcast a [1, F] AP across n partitions (partition step 0)."""
    return bass.AP(tensor=ap.tensor, offset=ap.offset, ap=[[0, n]] + list(ap.ap[1:]))


DEBUG_L1 = False


def build_launch1():
    nc = bacc.Bacc("TRN2", target_bir_lowering=False, debug=False, num_devices=NCORES)
    q = nc.dram_tensor("q", [S, D], F32, kind="ExternalInput")
    k = nc.dram_tensor("k", [S, D], F32, kind="ExternalInput")
    v = nc.dram_tensor("v", [S, D], F32, kind="ExternalInput")
    wq = nc.dram_tensor("wq", [D, FPC], F32, kind="ExternalInput")
    wk = nc.dram_tensor("wk", [D, FPC], F32, kind="ExternalInput")
    wv = nc.dram_tensor("wv", [D, FPC], F32, kind="ExternalInput")
    bqv = nc.dram_tensor("bqv", [FPC], F32, kind="ExternalInput")
    bkv = nc.dram_tensor("bkv", [FPC], F32, kind="ExternalInput")
    bvv = nc.dram_tensor("bvv", [FPC], F32, kind="ExternalInput")
    relt = nc.dram_tensor("relt", [REL_ROWS, REL_DIM := 64], F32, kind="ExternalInput")
    attn_out = nc.dram_tensor("attn_out", [FPC, S], BF16, kind="ExternalOutput")
    ebias_dram = nc.dram_tensor("ebias_dram", [EB_LEN], F32)

    with tile.TileContext(nc) as tc:
        with (
            tc.tile_pool(name="singles", bufs=1) as singles,
            tc.tile_pool(name="wstage", bufs=1) as wstage,
            tc.tile_pool(name="stage", bufs=3) as stage,
            tc.tile_pool(name="xtp", bufs=3) as xtp,
            tc.tile_pool(name="small", bufs=1) as small,
            tc.tile_pool(name="ptile", bufs=4) as ptile,
            tc.tile_pool(name="attnsb", bufs=1) as attnsb,
            tc.tile_pool(name="psum_tp", bufs=2, space="PSUM") as psum_tp,
            tc.tile_pool(name="psum_proj", bufs=2, space="PSUM") as psum_proj,
            tc.tile_pool(name="psum_att", bufs=2, space="PSUM") as psum_att,
        ):
            # ---------------- constants / weights ----------------
            ident = singles.tile([128, 128], BF16, name="ident")
            make_identity(nc, ident)

            wq_b = singles.tile([128, 8, FPC], BF16, name="wq_b")
            wk_b = singles.tile([128, 8, FPC], BF16, name="wk_b")
            wv_b = singles.tile([128, 8, FPC], BF16, name="wv_b")
            for wb, wd, nm in ((wq_b, wq, "q"), (wk_b, wk, "k"), (wv_b, wv, "v")):
                wf = wstage.tile([128, 8, FPC], F32, tag="wf", name=f"wf_{nm}")
                nc.sync.dma_start(out=wf, in_=wd.ap().rearrange("(c p) f -> p c f", p=128))
                nc.vector.tensor_copy(out=wb, in_=wf)

            # biases: feature-chunk layout [128, 2] (for QT/KT, per-partition)
            bq_s = singles.tile([128, 2], F32, name="bq_s")
            bk_s = singles.tile([128, 2], F32, name="bk_s")
            nc.sync.dma_start(out=bq_s, in_=bqv.ap().rearrange("(c p) -> p c", p=128))
            nc.sync.dma_start(out=bk_s, in_=bkv.ap().rearrange("(c p) -> p c", p=128))
            bq_sc = singles.tile([128, 2], F32, name="bq_sc")
            nc.vector.tensor_scalar_mul(bq_sc, bq_s, 0.125)
            # bv replicated across partitions [128, 256] (varies along free dim)
            bv_rep = singles.tile([128, FPC], F32, name="bv_rep")
            nc.gpsimd.dma_start(
                out=bv_rep,
                in_=bass.AP(tensor=bvv.ap().tensor, offset=0, ap=[[0, 128], [1, FPC]]),
            )

            # ---------------- ebias ----------------
            rsum = small.tile([128, 16], F32, name="rsum")
            for i in range(REL_ROWS // 128):
                rt = stage.tile([128, REL_DIM], F32, tag="relrows", name="rt")
                nc.sync.dma_start(out=rt, in_=relt.ap()[i * 128 : (i + 1) * 128, :])
                nc.vector.tensor_reduce(
                    out=rsum[:, i : i + 1],
                    in_=rt,
                    axis=mybir.AxisListType.X,
                    op=mybir.AluOpType.add,
                )
            ebias_sb = small.tile([128, 16], F32, name="ebias_sb")
            nc.scalar.activation(out=ebias_sb, in_=rsum, func=AF.Exp)
            ebias_tail = small.tile([128, 4], F32, name="ebias_tail")
            nc.vector.memset(ebias_tail, float(np.exp(-30.0)))
            nc.sync.dma_start(
                out=ebias_dram.ap()[0:REL_ROWS].rearrange("(c p) -> p c", p=128),
                in_=ebias_sb,
            )
            nc.sync.dma_start(
                out=ebias_dram.ap()[REL_ROWS:EB_LEN].rearrange("(c p) -> p c", p=128),
                in_=ebias_tail,
            )
            # ebufT[p, y] = ebias[p + y]; reverse once on-chip so per-tile
            # slices use positive unit strides: ebr[p, y] = ebias[p + 2431 - y]
            ebufT = singles.tile([128, EB_W], BF16, name="ebufT")
            nc.gpsimd.dma_start(
                out=ebufT,
                in_=bass.AP(
                    tensor=ebias_dram.ap().tensor, offset=0, ap=[[1, 128], [1, EB_W]]
                ),
            )
            ebr = singles.tile([128, EB_W], BF16, name="ebr")
            _last = ebufT[:, EB_W - 1 : EB_W]
            nc.vector.tensor_copy(
                out=ebr,
                in_=bass.AP(
                    tensor=_last.tensor,
                    offset=_last.offset,
                    ap=[list(_last.ap[0]), [-1, EB_W]],
                ),
            )

            # ---------------- projections ----------------
            # QT/KT: [128, pair(2), S] bf16; pair p rows 0:64 = head 2p,
            # rows 64:128 = head 2p+1 (so lhsT/rhs share base partitions).
            qT_sb = attnsb.tile([128, 2, S], BF16, name="qT_sb")
            kT_sb = attnsb.tile([128, 2, S], BF16, name="kT_sb")
            # V token-major with ones columns: [128, tok-tile(16), 260]
            # pair p cols p*130: [V_h(64) | 1 | V_h'(64) | 1]
            v_sb = attnsb.tile([128, NKC, 260], BF16, name="v_sb")
            nc.vector.memset(v_sb, 1.0)
            ones_col = singles.tile([1, 64], BF16, name="ones_col")
            nc.vector.memset(ones_col, 1.0)

            for src, kind in ((q, "q"), (k, "k"), (v, "v")):
                for tg in range(NTT // 4):  # 512-token groups
                    # xtg[:, c, :] = x^T d-chunk c for the group's 512 tokens
                    xtg = xtp.tile([128, 8, 512], BF16, tag="xtg", name="xtg")
                    for t4 in range(4):
                        t = tg * 4 + t4
                        xb = stage.tile([128, D], BF16, tag="xb", name="xb")
                        # SWDGE cast DMA fp32 -> bf16
                        nc.gpsimd.dma_start(
                            out=xb, in_=src.ap()[t * 128 : (t + 1) * 128, :]
                        )
                        for cg in range(2):  # 4 d-chunks per PSUM tile
                            pstp = psum_tp.tile(
                                [128, 512], BF16, tag="pstp", name="pstp"
                            )
                            for c4 in range(4):
                                c = cg * 4 + c4
                                nc.tensor.transpose(
                                    pstp[:, c4 * 128 : (c4 + 1) * 128],
                                    xb[:, c * 128 : (c + 1) * 128],
                                    ident,
                                )
                            # one batched evac, strided into xtg
                            nc.vector.tensor_copy(
                                out=xtg[:, cg * 4 : cg * 4 + 4, t4 * 128 : (t4 + 1) * 128],
                                in_=pstp[:, :].rearrange("p (c f) -> p c f", c=4),
                            )
                        if kind == "v":
                            pv = psum_proj.tile([128, FPC], F32, tag="proj", name="pv")
                            for c in range(8):
                                nc.tensor.matmul(
                                    pv,
                                    lhsT=xtg[:, c, t4 * 128 : (t4 + 1) * 128],
                                    rhs=wv_b[:, c, :],
                                    start=(c == 0),
                                    stop=(c == 7),
                                )
                            for h in range(HPC):
                                pair, par = divmod(h, 2)
                                off = pair * 130 + par * 65
                                nc.vector.tensor_add(
                                    v_sb[:, t, off : off + 64],
                                    pv[:, h * 64 : (h + 1) * 64],
                                    bv_rep[:, h * 64 : (h + 1) * 64],
                                )
                    if kind != "v":
                        dst = qT_sb if kind == "q" else kT_sb
                        wb = wq_b if kind == "q" else wk_b
                        bias = bq_sc if kind == "q" else bk_s
                        scale = 0.125 if kind == "q" else 1.0
                        for m in range(2):
                            pq = psum_proj.tile([128, 512], F32, tag="proj", name="pq")
                            for c in range(8):
                                nc.tensor.matmul(
                                    pq,
                                    lhsT=wb[:, c, m * 128 : (m + 1) * 128],
                                    rhs=xtg[:, c, :],
                                    start=(c == 0),
                                    stop=(c == 7),
                                )
                            # rows m*128+p = heads 2m..2m+1 -> pair m directly
                            nc.scalar.activation(
                                out=dst[:, m, tg * 512 : (tg + 1) * 512],
                                in_=pq,
                                func=AF.Identity,
                                bias=bias[:, m : m + 1],
                                scale=scale,
                            )

            if DEBUG_L1:
                dbg_qt = nc.dram_tensor("dbg_qt", [128, 2, S], BF16, kind="ExternalOutput")
                dbg_kt = nc.dram_tensor("dbg_kt", [128, 2, S], BF16, kind="ExternalOutput")
                dbg_v = nc.dram_tensor("dbg_v", [128, NKC, 260], BF16, kind="ExternalOutput")
                dbg_eb = nc.dram_tensor("dbg_eb", [128, EB_W], BF16, kind="ExternalOutput")
                nc.sync.dma_start(out=dbg_qt.ap(), in_=qT_sb[:, :, :])
                nc.sync.dma_start(out=dbg_kt.ap(), in_=kT_sb[:, :, :])
                nc.sync.dma_start(out=dbg_v.ap(), in_=v_sb[:, :, :])
                nc.sync.dma_start(out=dbg_eb.ap(), in_=ebr[:, :])

            # ---------------- attention ----------------
            for t in range(NQT):
                q0 = t * QTILE
                nkc = 4 * t + 4
                nums = []
                # all four heads' denominators parked on partition 0
                dens = small.tile([1, HPC, QTILE], F32, tag="dens", name="dens")
                for h in range(HPC):
                    pair, par = divmod(h, 2)
                    po = par * 64
                    voff = pair * 130 + par * 65
                    pv_ps = psum_att.tile([128, QTILE], F32, tag="pvps", name="pv_ps")
                    for kc in range(nkc):
                        st_ps = psum_att.tile(
                            [128, QTILE], F32, tag="stps", name="st_ps"
                        )
                        nc.tensor.matmul(
                            st_ps,
                            lhsT=kT_sb[po : po + 64, pair, kc * 128 : (kc + 1) * 128],
                            rhs=qT_sb[po : po + 64, pair, q0 : q0 + QTILE],
                            start=True,
                            stop=True,
                        )
                        pe = ptile.tile([128, QTILE], BF16, tag="pe", name="pe_t")
                        nc.scalar.activation(out=pe, in_=st_ps, func=AF.Exp)
                        pb = ptile.tile([128, QTILE], BF16, tag="pb", name="pb_t")
                        y0 = q0 - 128 * kc + 384
                        nc.vector.tensor_mul(pb, pe, ebr[:, y0 : y0 + QTILE])
                        nc.tensor.matmul(
                            pv_ps[0:65],
                            lhsT=v_sb[:, kc, voff : voff + 65],
                            rhs=pb,
                            start=(kc == 0),
                            stop=(kc == nkc - 1),
                        )
                    # evacuate numerator + denominator, free PSUM quickly
                    num = ptile.tile([64, QTILE], BF16, tag="num", name="num")
                    nc.vector.tensor_copy(out=num, in_=pv_ps[0:64])
                    nums.append(num)
                    nc.vector.tensor_copy(out=dens[0:1, h, :], in_=pv_ps[64:65])
                # one batched reciprocal for all 4 heads
                recs = small.tile([1, HPC, QTILE], F32, tag="recs", name="recs")
                nc.vector.reciprocal(out=recs, in_=dens)
                recs_row = small.tile([1, HPC, QTILE], BF16, tag="recsrow", name="recs_row")
                nc.vector.tensor_copy(out=recs_row, in_=recs)
                for h in range(HPC):
                    # replicate recip to 64 partitions via ones x rec matmul
                    rec_ps = psum_att.tile([128, QTILE], F32, tag="stps", name="rec_ps")
                    nc.tensor.matmul(
                        rec_ps[0:64],
                        lhsT=ones_col,
                        rhs=recs_row[0:1, h, :],
                        start=True,
                        stop=True,
                    )
                    outt = ptile.tile([64, QTILE], BF16, tag="outt", name="outt")
                    nc.vector.tensor_mul(outt, nums[h], rec_ps[0:64])
                    nc.sync.dma_start(
                        out=attn_out.ap()[h * 64 : (h + 1) * 64, q0 : q0 + QTILE],
                        in_=outt,
                    )
    nc.compile()
    return nc


def build_launch2():
    nc = bacc.Bacc("TRN2", target_bir_lowering=False, debug=False, num_devices=NCORES)
    attnT = nc.dram_tensor("attnT", [D, QTILE], BF16, kind="ExternalInput")
    wo = nc.dram_tensor("wo", [D, D], F32, kind="ExternalInput")
    bo = nc.dram_tensor("bo", [D], F32, kind="ExternalInput")
    out = nc.dram_tensor("out", [QTILE, D], F32, kind="ExternalOutput")

    with tile.TileContext(nc) as tc:
        with (
            tc.tile_pool(name="singles", bufs=1) as singles,
            tc.tile_pool(name="ps", bufs=4, space="PSUM") as psp,
            tc.tile_pool(name="sb", bufs=4) as sbp,
        ):
            wo_b = singles.tile([128, 8, D], BF16, name="wo_b")
            nc.gpsimd.dma_start(
                out=wo_b, in_=wo.ap().rearrange("(c p) f -> p c f", p=128)
            )
            at = singles.tile([128, 8, QTILE], BF16, name="at")
            nc.sync.dma_start(
                out=at, in_=attnT.ap().rearrange("(c p) t -> p c t", p=128)
            )
            bo_rep = singles.tile([128, D], F32, name="bo_rep")
            nc.gpsimd.dma_start(
                out=bo_rep,
                in_=bass.AP(tensor=bo.ap().tensor, offset=0, ap=[[0, 128], [1, D]]),
            )
            for m in range(QTILE // 128):
                for n in range(2):
                    ps = psp.tile([128, 512], F32, tag="ps", name="ps")
                    for c in range(8):
                        nc.tensor.matmul(
                            ps,
                            lhsT=at[:, c, m * 128 : (m + 1) * 128],
                            rhs=wo_b[:, c, n * 512 : (n + 1) * 512],
                            start=(c == 0),
                            stop=(c == 7),
                        )
                    ob = sbp.tile([128, 512], F32, tag="ob", name="ob")
                    nc.vector.tensor_add(ob, ps, bo_rep[:, n * 512 : (n + 1) * 512])
                    nc.sync.dma_start(
                        out=out.ap()[m * 128 : (m + 1) * 128, n * 512 : (n + 1) * 512],
                        in_=ob,
                    )
    nc.compile()
    return nc


_NC1 = None
_NC2 = None
_last_inmaps = None


def _get_ncs():
    global _NC1, _NC2
    if _NC1 is None:
        _NC1 = build_launch1()
        _NC2 = build_launch2()
    return _NC1, _NC2


def kernel(q, k, v, Wq, bq, Wk, bk, Wv, bv, Wo, bo, rel_table, mask, **kw):
    q = np.asarray(q, np.float32)
    k = np.asarray(k, np.float32)
    v = np.asarray(v, np.float32)
    Wq = np.asarray(Wq, np.float32)
    Wk = np.asarray(Wk, np.float32)
    Wv = np.asarray(Wv, np.float32)
    Wo = np.asarray(Wo, np.float32)
    bq = np.asarray(bq, np.float32)
    bk = np.asarray(bk, np.float32)
    bv = np.asarray(bv, np.float32)
    bo = np.asarray(bo, np.float32)
    rel_table = np.asarray(rel_table, np.float32)

    nc1, nc2 = _get_ncs()
    relt = np.ascontiguousarray(rel_table[:REL_ROWS])

    in_maps = []
    for c in range(NCORES):
        b, hg = divmod(c, 4)
        fs = slice(hg * FPC, (hg + 1) * FPC)
        in_maps.append(
            {
                "q": np.ascontiguousarray(q[b]),
                "k": np.ascontiguousarray(k[b]),
                "v": np.ascontiguousarray(v[b]),
                "wq": np.ascontiguousarray(Wq[:, fs]),
                "wk": np.ascontiguousarray(Wk[:, fs]),
                "wv": np.ascontiguousarray(Wv[:, fs]),
                "bqv": np.ascontiguousarray(bq[fs]),
                "bkv": np.ascontiguousarray(bk[fs]),
                "bvv": np.ascontiguousarray(bv[fs]),
                "relt": relt,
            }
        )
    res1 = run_bass_kernel_spmd(nc1, in_maps, core_ids=list(range(NCORES)))
    attnT = [
        np.concatenate([res1.results[b * 4 + hg]["attn_out"] for hg in range(4)], 0)
        for b in range(B)
    ]
    in_maps2 = []
    for c in range(NCORES):
        b, tq = divmod(c, 4)
        in_maps2.append(
            {
                "attnT": np.ascontiguousarray(attnT[b][:, tq * QTILE : (tq + 1) * QTILE]),
                "wo": Wo,
                "bo": bo,
            }
        )
    res2 = run_bass_kernel_spmd(nc2, in_maps2, core_ids=list(range(NCORES)))
    global _last_inmaps
    _last_inmaps = (in_maps, in_maps2)
    out = np.empty((B, S, D), np.float32)
    for c in range(NCORES):
        b, tq = divmod(c, 4)
        out[b, tq * QTILE : (tq + 1) * QTILE] = res2.results[c]["out"]
    return out


# revision 30
# speedup vs baseline: 1.1726x; 1.1726x over previous
"""Trainium2 Bass kernel for nn_MultiHeadAttention_35570919146065 (B=2, S=2048,
D=1024, H=16, causal + relative-position bias).

Sharding (8 NeuronCores):
  launch 1: core c = (batch c//4, heads 4*(c%4)..): QKV projections +
            attention, emits attnT [256, 2048] bf16 per core.
  launch 2: core c = (batch c//4, token block c%4 of 512): out projection.

All matmuls bf16 with fp32 PSUM accumulation. Relative-position bias and the
causal mask are folded into one multiplicative term: P = exp(S^T) * ebufT,
where ebias[x] = exp(sum_d rel_table[x, d]) for x <= 2047 and exp(-30) ~= 0
for x > 2047 (masked), x = j - i + 2047. Softmax denominators come from an
appended ones-column in the PV matmul; normalization is deferred to the small
attnT tile.
"""

import numpy as np

import concourse.bass as bass
import concourse.mybir as mybir
import concourse.tile as tile
from concourse import bacc
from concourse.bass_utils import run_bass_kernel_spmd
from concourse.masks import make_identity

B, S, D, H = 2, 2048, 1024, 16
DK = D // H
NCORES = 8
HPC = 4  # heads per core
FPC = HPC * DK  # 256 features per core

QTILE = 512
NQT = S // QTILE
NKC = S // 128
NTT = S // 128  # token tiles

EB_W = 2432  # ebufT free size
EB_LEN = 2560  # ebias length
REL_ROWS = 2048  # rel rows needed (x <= 2047)

DT = mybir.dt
F32 = DT.float32
BF16 = DT.bfloat16
AF = mybir.ActivationFunctionType


def row_bcast(ap, n):
    """Broadcast a [1, F] AP across n partitions (partition step 0)."""
    return bass.AP(tensor=ap.tensor, offset=ap.offset, ap=[[0, n]] + list(ap.ap[1:]))


DEBUG_L1 = False


def build_launch1():
    nc = bacc.Bacc("TRN2", target_bir_lowering=False, debug=False, num_devices=NCORES)
    q = nc.dram_tensor("q", [S, D], F32, kind="ExternalInput")
    k = nc.dram_tensor("k", [S, D], F32, kind="ExternalInput")
    v = nc.dram_tensor("v", [S, D], F32, kind="ExternalInput")
    wq = nc.dram_tensor("wq", [D, FPC], F32, kind="ExternalInput")
    wk = nc.dram_tensor("wk", [D, FPC], F32, kind="ExternalInput")
    wv = nc.dram_tensor("wv", [D, FPC], F32, kind="ExternalInput")
    bqv = nc.dram_tensor("bqv", [FPC], F32, kind="ExternalInput")
    bkv = nc.dram_tensor("bkv", [FPC], F32, kind="ExternalInput")
    bvv = nc.dram_tensor("bvv", [FPC], F32, kind="ExternalInput")
    relt = nc.dram_tensor("relt", [REL_ROWS, REL_DIM := 64], F32, kind="ExternalInput")
    attn_out = nc.dram_tensor("attn_out", [FPC, S], BF16, kind="ExternalOutput")
    ebias_dram = nc.dram_tensor("ebias_dram", [EB_LEN], F32)

    with tile.TileContext(nc) as tc:
        with (
            tc.tile_pool(name="singles", bufs=1) as singles,
            tc.tile_pool(name="wstage", bufs=1) as wstage,
            tc.tile_pool(name="stage", bufs=3) as stage,
            tc.tile_pool(name="xtp", bufs=3) as xtp,
            tc.tile_pool(name="small", bufs=1) as small,
            tc.tile_pool(name="ptile", bufs=4) as ptile,
            tc.tile_pool(name="attnsb", bufs=1) as attnsb,
            tc.tile_pool(name="psum_tp", bufs=2, space="PSUM") as psum_tp,
            tc.tile_pool(name="psum_proj", bufs=2, space="PSUM") as psum_proj,
            tc.tile_pool(name="psum_att", bufs=2, space="PSUM") as psum_att,
        ):
            # ---------------- constants / weights ----------------
            ident = singles.tile([128, 128], BF16, name="ident")
            make_identity(nc, ident)

            wq_b = singles.tile([128, 8, FPC], BF16, name="wq_b")
            wk_b = singles.tile([128, 8, FPC], BF16, name="wk_b")
            wv_b = singles.tile([128, 8, FPC], BF16, name="wv_b")
            for wb, wd, nm in ((wq_b, wq, "q"), (wk_b, wk, "k"), (wv_b, wv, "v")):
                wf = wstage.tile([128, 8, FPC], F32, tag="wf", name=f"wf_{nm}")
                nc.sync.dma_start(out=wf, in_=wd.ap().rearrange("(c p) f -> p c f", p=128))
                nc.vector.tensor_copy(out=wb, in_=wf)

            # biases: feature-chunk layout [128, 2] (for QT/KT, per-partition)
            bq_s = singles.tile([128, 2], F32, name="bq_s")
            bk_s = singles.tile([128, 2], F32, name="bk_s")
            nc.sync.dma_start(out=bq_s, in_=bqv.ap().rearrange("(c p) -> p c", p=128))
            nc.sync.dma_start(out=bk_s, in_=bkv.ap().rearrange("(c p) -> p c", p=128))
            bq_sc = singles.tile([128, 2], F32, name="bq_sc")
            nc.vector.tensor_scalar_mul(bq_sc, bq_s, 0.125)
            # bv replicated across partitions [128, 256] (varies along free dim)
            bv_rep = singles.tile([128, FPC], F32, name="bv_rep")
            nc.gpsimd.dma_start(
                out=bv_rep,
                in_=bass.AP(tensor=bvv.ap().tensor, offset=0, ap=[[0, 128], [1, FPC]]),
            )

            # ---------------- ebias ----------------
            rsum = small.tile([128, 16], F32, name="rsum")
            for i in range(REL_ROWS // 128):
                rt = stage.tile([128, REL_DIM], F32, tag="relrows", name="rt")
                nc.sync.dma_start(out=rt, in_=relt.ap()[i * 128 : (i + 1) * 128, :])
                nc.vector.tensor_reduce(
                    out=rsum[:, i : i + 1],
                    in_=rt,
                    axis=mybir.AxisListType.X,
                    op=mybir.AluOpType.add,
                )
            ebias_sb = small.tile([128, 16], F32, name="ebias_sb")
            nc.scalar.activation(out=ebias_sb, in_=rsum, func=AF.Exp)
            ebias_tail = small.tile([128, 4], F32, name="ebias_tail")
            nc.vector.memset(ebias_tail, float(np.exp(-30.0)))
            nc.sync.dma_start(
                out=ebias_dram.ap()[0:REL_ROWS].rearrange("(c p) -> p c", p=128),
                in_=ebias_sb,
            )
            nc.sync.dma_start(
                out=ebias_dram.ap()[REL_ROWS:EB_LEN].rearrange("(c p) -> p c", p=128),
                in_=ebias_tail,
            )
            # ebufT[p, y] = ebias[p + y]; reverse once on-chip so per-tile
            # slices use positive unit strides: ebr[p, y] = ebias[p + 2431 - y]
            ebufT = singles.tile([128, EB_W], BF16, name="ebufT")
            nc.gpsimd.dma_start(
                out=ebufT,
                in_=bass.AP(
                    tensor=ebias_dram.ap().tensor, offset=0, ap=[[1, 128], [1, EB_W]]
                ),
            )
            ebr = singles.tile([128, EB_W], BF16, name="ebr")
            _last = ebufT[:, EB_W - 1 : EB_W]
            nc.vector.tensor_copy(
                out=ebr,
                in_=bass.AP(
                    tensor=_last.tensor,
                    offset=_last.offset,
                    ap=[list(_last.ap[0]), [-1, EB_W]],
                ),
            )

            # ---------------- projections ----------------
            # QT/KT: [128, pair(2), S] bf16; pair p rows 0:64 = head 2p,
            # rows 64:128 = head 2p+1 (so lhsT/rhs share base partitions).
            qT_sb = attnsb.tile([128, 2, S], BF16, name="qT_sb")
            kT_sb = attnsb.tile([128, 2, S], BF16, name="kT_sb")
            # V token-major with ones columns: [128, tok-tile(16), 260]
            # pair p cols p*130: [V_h(64) | 1 | V_h'(64) | 1]
            v_sb = attnsb.tile([128, NKC, 260], BF16, name="v_sb")
            nc.vector.memset(v_sb, 1.0)
            ones_col = singles.tile([1, 64], BF16, name="ones_col")
            nc.vector.memset(ones_col, 1.0)

            for src, kind in ((q, "q"), (k, "k"), (v, "v")):
                for tg in range(NTT // 4):  # 512-token groups
                    # xtg[:, c, :] = x^T d-chunk c for the group's 512 tokens
                    xtg = xtp.tile([128, 8, 512], BF16, tag="xtg", name="xtg")
                    for t4 in range(4):
                        t = tg * 4 + t4
                        xb = stage.tile([128, D], BF16, tag="xb", name="xb")
                        # SWDGE cast DMA fp32 -> bf16
                        nc.gpsimd.dma_start(
                            out=xb, in_=src.ap()[t * 128 : (t + 1) * 128, :]
                        )
                        for cg in range(2):  # 4 d-chunks per PSUM tile
                            pstp = psum_tp.tile(
                                [128, 512], BF16, tag="pstp", name="pstp"
                            )
                            for c4 in range(4):
                                c = cg * 4 + c4
                                nc.tensor.transpose(
                                    pstp[:, c4 * 128 : (c4 + 1) * 128],
                                    xb[:, c * 128 : (c + 1) * 128],
                                    ident,
                                )
                            # one batched evac, strided into xtg (on ACT; DVE
                            # is the bottleneck engine)
                            nc.scalar.activation(
                                out=xtg[:, cg * 4 : cg * 4 + 4, t4 * 128 : (t4 + 1) * 128],
                                in_=pstp[:, :].rearrange("p (c f) -> p c f", c=4),
                                func=AF.Copy,
                            )
                        if kind == "v":
                            pv = psum_proj.tile([128, FPC], F32, tag="proj", name="pv")
                            for c in range(8):
                                nc.tensor.matmul(
                                    pv,
                                    lhsT=xtg[:, c, t4 * 128 : (t4 + 1) * 128],
                                    rhs=wv_b[:, c, :],
                                    start=(c == 0),
                                    stop=(c == 7),
                                )
                            for h in range(HPC):
                                pair, par = divmod(h, 2)
                                off = pair * 130 + par * 65
                                nc.vector.tensor_add(
                                    v_sb[:, t, off : off + 64],
                                    pv[:, h * 64 : (h + 1) * 64],
                                    bv_rep[:, h * 64 : (h + 1) * 64],
                                )
                    if kind != "v":
                        dst = qT_sb if kind == "q" else kT_sb
                        wb = wq_b if kind == "q" else wk_b
                        bias = bq_sc if kind == "q" else bk_s
                        scale = 0.125 if kind == "q" else 1.0
                        for m in range(2):
                            pq = psum_proj.tile([128, 512], F32, tag="proj", name="pq")
                            for c in range(8):
                                nc.tensor.matmul(
                                    pq,
                                    lhsT=wb[:, c, m * 128 : (m + 1) * 128],
                                    rhs=xtg[:, c, :],
                                    start=(c == 0),
                                    stop=(c == 7),
                                )
                            # rows m*128+p = heads 2m..2m+1 -> pair m directly
                            nc.scalar.activation(
                                out=dst[:, m, tg * 512 : (tg + 1) * 512],
                                in_=pq,
                                func=AF.Identity,
                                bias=bias[:, m : m + 1],
                                scale=scale,
                            )

            if DEBUG_L1:
                dbg_qt = nc.dram_tensor("dbg_qt", [128, 2, S], BF16, kind="ExternalOutput")
                dbg_kt = nc.dram_tensor("dbg_kt", [128, 2, S], BF16, kind="ExternalOutput")
                dbg_v = nc.dram_tensor("dbg_v", [128, NKC, 260], BF16, kind="ExternalOutput")
                dbg_eb = nc.dram_tensor("dbg_eb", [128, EB_W], BF16, kind="ExternalOutput")
                nc.sync.dma_start(out=dbg_qt.ap(), in_=qT_sb[:, :, :])
                nc.sync.dma_start(out=dbg_kt.ap(), in_=kT_sb[:, :, :])
                nc.sync.dma_start(out=dbg_v.ap(), in_=v_sb[:, :, :])
                nc.sync.dma_start(out=dbg_eb.ap(), in_=ebr[:, :])

            # ---------------- attention ----------------
            for t in range(NQT):
                q0 = t * QTILE
                nkc = 4 * t + 4
                nums = []
                # all four heads' denominators parked on partition 0
                dens = small.tile([1, HPC, QTILE], F32, tag="dens", name="dens")
                for h in range(HPC):
                    pair, par = divmod(h, 2)
                    po = par * 64
                    voff = pair * 130 + par * 65
                    pv_ps = psum_att.tile([128, QTILE], F32, tag="pvps", name="pv_ps")
                    for kc in range(nkc):
                        st_ps = psum_att.tile(
                            [128, QTILE], F32, tag="stps", name="st_ps"
                        )
                        nc.tensor.matmul(
                            st_ps,
                            lhsT=kT_sb[po : po + 64, pair, kc * 128 : (kc + 1) * 128],
                            rhs=qT_sb[po : po + 64, pair, q0 : q0 + QTILE],
                            start=True,
                            stop=True,
                        )
                        pe = ptile.tile([128, QTILE], BF16, tag="pe", name="pe_t")
                        nc.scalar.activation(out=pe, in_=st_ps, func=AF.Exp)
                        pb = ptile.tile([128, QTILE], BF16, tag="pb", name="pb_t")
                        y0 = q0 - 128 * kc + 384
                        nc.vector.tensor_mul(pb, pe, ebr[:, y0 : y0 + QTILE])
                        nc.tensor.matmul(
                            pv_ps[0:65],
                            lhsT=v_sb[:, kc, voff : voff + 65],
                            rhs=pb,
                            start=(kc == 0),
                            stop=(kc == nkc - 1),
                        )
                    # evacuate numerator + denominator, free PSUM quickly
                    num = ptile.tile([64, QTILE], BF16, tag="num", name="num")
                    nc.vector.tensor_copy(out=num, in_=pv_ps[0:64])
                    nums.append(num)
                    nc.vector.tensor_copy(out=dens[0:1, h, :], in_=pv_ps[64:65])
                # one batched reciprocal for all 4 heads
                recs = small.tile([1, HPC, QTILE], F32, tag="recs", name="recs")
                nc.vector.reciprocal_approx_fast(out=recs, in_=dens)
                recs_row = small.tile([1, HPC, QTILE], BF16, tag="recsrow", name="recs_row")
                nc.vector.tensor_copy(out=recs_row, in_=recs)
                for h in range(HPC):
                    # replicate recip to 64 partitions via ones x rec matmul
                    rec_ps = psum_att.tile([128, QTILE], F32, tag="stps", name="rec_ps")
                    nc.tensor.matmul(
                        rec_ps[0:64],
                        lhsT=ones_col,
                        rhs=recs_row[0:1, h, :],
                        start=True,
                        stop=True,
                    )
                    outt = ptile.tile([64, QTILE], BF16, tag="outt", name="outt")
                    nc.vector.tensor_mul(outt, nums[h], rec_ps[0:64])
                    nc.sync.dma_start(
                        out=attn_out.ap()[h * 64 : (h + 1) * 64, q0 : q0 + QTILE],
                        in_=outt,
                    )
    nc.compile()
    return nc


def build_launch2():
    nc = bacc.Bacc("TRN2", target_bir_lowering=False, debug=False, num_devices=NCORES)
    attnT = nc.dram_tensor("attnT", [D, QTILE], BF16, kind="ExternalInput")
    wo = nc.dram_tensor("wo", [D, D], F32, kind="ExternalInput")
    bo = nc.dram_tensor("bo", [D], F32, kind="ExternalInput")
    out = nc.dram_tensor("out", [QTILE, D], F32, kind="ExternalOutput")

    with tile.TileContext(nc) as tc:
        with (
            tc.tile_pool(name="singles", bufs=1) as singles,
            tc.tile_pool(name="ps", bufs=4, space="PSUM") as psp,
            tc.tile_pool(name="sb", bufs=4) as sbp,
        ):
            wo_b = singles.tile([128, 8, D], BF16, name="wo_b")
            nc.gpsimd.dma_start(
                out=wo_b, in_=wo.ap().rearrange("(c p) f -> p c f", p=128)
            )
            at = singles.tile([128, 8, QTILE], BF16, name="at")
            nc.sync.dma_start(
                out=at, in_=attnT.ap().rearrange("(c p) t -> p c t", p=128)
            )
            bo_rep = singles.tile([128, D], F32, name="bo_rep")
            nc.gpsimd.dma_start(
                out=bo_rep,
                in_=bass.AP(tensor=bo.ap().tensor, offset=0, ap=[[0, 128], [1, D]]),
            )
            for m in range(QTILE // 128):
                for n in range(2):
                    ps = psp.tile([128, 512], F32, tag="ps", name="ps")
                    for c in range(8):
                        nc.tensor.matmul(
                            ps,
                            lhsT=at[:, c, m * 128 : (m + 1) * 128],
                            rhs=wo_b[:, c, n * 512 : (n + 1) * 512],
                            start=(c == 0),
                            stop=(c == 7),
                        )
                    ob = sbp.tile([128, 512], F32, tag="ob", name="ob")
                    nc.vector.tensor_add(ob, ps, bo_rep[:, n * 512 : (n + 1) * 512])
                    nc.sync.dma_start(
                        out=out.ap()[m * 128 : (m + 1) * 128, n * 512 : (n + 1) * 512],
                        in_=ob,
                    )
    nc.compile()
    return nc


_NC1 = None
_NC2 = None
_last_inmaps = None


def _get_ncs():
    global _NC1, _NC2
    if _NC1 is None:
        _NC1 = build_launch1()
        _NC2 = build_launch2()
    return _NC1, _NC2


def kernel(q, k, v, Wq, bq, Wk, bk, Wv, bv, Wo, bo, rel_table, mask, **kw):
    q = np.asarray(q, np.float32)
    k = np.asarray(k, np.float32)
    v = np.asarray(v, np.float32)
    Wq = np.asarray(Wq, np.float32)
    Wk = np.asarray(Wk, np.float32)
    Wv = np.asarray(Wv, np.float32)
    Wo = np.asarray(Wo, np.float32)
    bq = np.asarray(bq, np.float32)
    bk = np.asarray(bk, np.float32)
    bv = np.asarray(bv, np.float32)
    bo = np.asarray(bo, np.float32)
    rel_table = np.asarray(rel_table, np.float32)

    nc1, nc2 = _get_ncs()
    relt = np.ascontiguousarray(rel_table[:REL_ROWS])

    in_maps = []
    for c in range(NCORES):
        b, hg = divmod(c, 4)
        fs = slice(hg * FPC, (hg + 1) * FPC)
        in_maps.append(
            {
                "q": np.ascontiguousarray(q[b]),
                "k": np.ascontiguousarray(k[b]),
                "v": np.ascontiguousarray(v[b]),
                "wq": np.ascontiguousarray(Wq[:, fs]),
                "wk": np.ascontiguousarray(Wk[:, fs]),
                "wv": np.ascontiguousarray(Wv[:, fs]),
                "bqv": np.ascontiguousarray(bq[fs]),
                "bkv": np.ascontiguousarray(bk[fs]),
                "bvv": np.ascontiguousarray(bv[fs]),
                "relt": relt,
            }
        )
    res1 = run_bass_kernel_spmd(nc1, in_maps, core_ids=list(range(NCORES)))
    attnT = [
        np.concatenate([res1.results[b * 4 + hg]["attn_out"] for hg in range(4)], 0)
        for b in range(B)
    ]
    in_maps2 = []
    for c in range(NCORES):
        b, tq = divmod(c, 4)
        in_maps2.append(
            {
                "attnT": np.ascontiguousarray(attnT[b][:, tq * QTILE : (tq + 1) * QTILE]),
                "wo": Wo,
                "bo": bo,
            }
        )
    res2 = run_bass_kernel_spmd(nc2, in_maps2, core_ids=list(range(NCORES)))
    global _last_inmaps
    _last_inmaps = (in_maps, in_maps2)
    out = np.empty((B, S, D), np.float32)
    for c in range(NCORES):
        b, tq = divmod(c, 4)
        out[b, tq * QTILE : (tq + 1) * QTILE] = res2.results[c]["out"]
    return out
